# revision 1
# baseline (speedup 1.0000x reference)
"""GCN layer (message passing) on 8 Trainium2 NeuronCores.

out = relu(((D^-1/2 A D^-1/2) X) @ W.T) + X

Strategy (dst-sharded graph partitioning):
  - Destination nodes sharded across 8 cores (12500 nodes each); every core
    holds the full feature table (random-access gather source) and computes
    its 12500 output rows; the host concatenates.
  - Host-side prep (index-space only): per-edge weight ns2 = norm[src]*norm[dst]
    (both rsqrt-degree norms folded into the edge weight); edges grouped by
    (dst tile of 128 nodes, src bucket of 25000 nodes, src) so each dst tile's
    sources are gathered with dma_gather (int16 indices => src buckets), with
    ascending addresses per stream for HBM locality.
  - Device, per dst tile: up to 4 dma_gather calls pull all edge source rows
    into X (the dominant memory traffic ~216MB/core). The segment-sum runs on
    the tensor engine as  zT[i,d] += X_c[e,i].T @ S_c[e,d]  where
    S_c[e,d] = (d == local_dst[e]) * ns2[e] is built with one fused
    tensor_scalar (is_equal then mult) against a constant iota row matrix.
    Then y[d,o] = zT.T @ W.T on the PE, ReLU on ACT, residual add on DVE.
  - num_idxs per gather is static per (tile, bucket) = max count over the 8
    cores (SPMD same-program constraint), so padding is only the cross-core
    spread (~5%); pad slots gather row 0 of the bucket and are annihilated by
    local_dst = -1 (one-hot row of zeros). Unwritten tail columns of X are
    killed the same way, but the first X pool slots are memzeroed once since
    0 * garbage-NaN would poison PSUM.
"""

import math

import numpy as np

import concourse.bacc as bacc
import concourse.mybir as mybir
from concourse.bass_utils import run_bass_kernel_spmd
from concourse.tile import TileContext

P = 128
N_CORES = 8
BUCKET_MAX = 25000  # int16 gather indices: bucket the node space


def _prepare(features, W, edge_src, edge_dst, n_cores=N_CORES, bucket_max=BUCKET_MAX):
    """Partition the graph by dst core / dst tile / src bucket."""
    features = np.asarray(features, dtype=np.float32)
    W = np.asarray(W, dtype=np.float32)
    edge_src = np.asarray(edge_src, dtype=np.int32)
    edge_dst = np.asarray(edge_dst, dtype=np.int32)

    n_nodes, d = features.shape
    assert d == P
    assert n_nodes % n_cores == 0
    npc = n_nodes // n_cores
    n_tiles = math.ceil(npc / P)
    rows_last = npc - (n_tiles - 1) * P
    nb = math.ceil(n_nodes / bucket_max)
    B = math.ceil(n_nodes / nb)
    assert B <= 32768

    degs = np.bincount(edge_dst, minlength=n_nodes).astype(np.float32)
    norm = 1.0 / np.sqrt(np.maximum(degs, 1.0), dtype=np.float32)
    ns2 = norm[edge_src] * norm[edge_dst]

    core_of = edge_dst // npc

    # first pass: per-core sorted edge lists and per-(tile,bucket) counts
    per_core = []
    counts_all = np.zeros((n_cores, n_tiles, nb), np.int64)
    for k in range(n_cores):
        sel = np.flatnonzero(core_of == k)
        src_k = edge_src[sel]
        ldst = edge_dst[sel] - k * npc
        tile_of = ldst // P
        bucket = src_k // B
        order = np.lexsort((src_k, bucket, tile_of))
        sel = sel[order]
        gid = tile_of[order] * nb + bucket[order]
        counts = np.bincount(gid, minlength=n_tiles * nb).reshape(n_tiles, nb)
        counts_all[k] = counts
        per_core.append((sel, gid, (ldst[order] % P).astype(np.float32)))

    # static per-(tile,bucket) gather sizes: max across cores
    n_tb = counts_all.max(axis=0)  # [n_tiles, nb]
    ct_tb = (n_tb + P - 1) // P  # chunks per (tile, bucket)
    C_t = ct_tb.sum(axis=1)  # chunks per tile
    icols_tb = (n_tb + 15) // 16  # int16 idx columns per (tile, bucket)
    icols_t = icols_tb.sum(axis=1)

    # column offsets in the packed DRAM arrays
    chunk_off_in_tile = np.cumsum(ct_tb, axis=1) - ct_tb  # [n_tiles, nb]
    icol_off_in_tile = np.cumsum(icols_tb, axis=1) - icols_tb
    ldns_col_off = np.concatenate([[0], np.cumsum(3 * C_t)])[:-1]  # per tile
    icol_off_tile = np.concatenate([[0], np.cumsum(icols_t)])[:-1]
    total_icols = int(icols_t.sum())
    total_ldns = int((3 * C_t).sum())

    layout = dict(
        n_nodes=n_nodes,
        npc=npc,
        n_tiles=n_tiles,
        rows_last=rows_last,
        nb=nb,
        B=B,
        n_tb=n_tb,
        ct_tb=ct_tb,
        C_t=C_t,
        icols_tb=icols_tb,
        chunk_off_in_tile=chunk_off_in_tile,
        icol_off_in_tile=icol_off_in_tile,
        ldns_col_off=ldns_col_off,
        icol_off_tile=icol_off_tile,
        total_icols=total_icols,
        total_ldns=total_ldns,
    )

    in_maps = []
    wt = np.ascontiguousarray(W.T)  # wt[i, o] = W[o, i]
    iotam = np.tile(np.arange(P, dtype=np.float32), (P, 1))
    for k in range(n_cores):
        sel, gid, ld_sorted = per_core[k]
        group_start = np.zeros(n_tiles * nb, np.int64)
        cnts = counts_all[k].reshape(-1)
        group_start[1:] = np.cumsum(cnts)[:-1]
        pos = np.arange(len(sel)) - group_start[gid]
        t_of = gid // nb
        b_of = gid % nb

        # idx array [16, total_icols] then replicated to 128 partitions
        idx16 = np.zeros((16, total_icols), np.int16)
        icol = icol_off_tile[t_of] + icol_off_in_tile[t_of, b_of] + pos // 16
        idx16[pos % 16, icol] = (edge_src[sel] - b_of * B).astype(np.int16)
        idxm = np.tile(idx16, (8, 1))

        # ldns array [128, total_ldns]: per tile [ld columns | ns columns]
        ldns = np.zeros((P, total_ldns), np.float32)
        # default ld = -1 in all ld column regions
        for t in range(n_tiles):
            ldns[:, ldns_col_off[t] : ldns_col_off[t] + C_t[t]] = -1.0
        cit = chunk_off_in_tile[t_of, b_of] + pos // P
        e_idx = pos % P
        ldns[e_idx, ldns_col_off[t_of] + cit] = ld_sorted
        ldns[e_idx, ldns_col_off[t_of] + C_t[t_of] + cit] = ns2[sel]
        ldns[e_idx, ldns_col_off[t_of] + 2 * C_t[t_of] + cit] = -ns2[sel]

        in_maps.append(
            {
                "feats": features,
                "idxm": np.ascontiguousarray(idxm),
                "ldns": np.ascontiguousarray(ldns),
                "wt": wt,
                "iotam": iotam,
                "resid": np.ascontiguousarray(features[k * npc : (k + 1) * npc]),
            }
        )
    return in_maps, layout


def _build_program(layout):
    f32 = mybir.dt.float32
    i16 = mybir.dt.int16
    n_nodes = layout["n_nodes"]
    npc = layout["npc"]
    n_tiles = layout["n_tiles"]
    rows_last = layout["rows_last"]
    nb = layout["nb"]
    B = layout["B"]
    n_tb = layout["n_tb"]
    ct_tb = layout["ct_tb"]
    C_t = layout["C_t"]
    icols_tb = layout["icols_tb"]
    chunk_off_in_tile = layout["chunk_off_in_tile"]
    icol_off_in_tile = layout["icol_off_in_tile"]
    ldns_col_off = layout["ldns_col_off"]
    icol_off_tile = layout["icol_off_tile"]
    Cmax = int(C_t.max())

    nc = bacc.Bacc(num_swdge_queues=4)
    feats = nc.declare_dram_parameter("feats", [n_nodes, P], f32, isOutput=False)
    idxm = nc.declare_dram_parameter(
        "idxm", [P, layout["total_icols"]], i16, isOutput=False
    )
    ldns = nc.declare_dram_parameter(
        "ldns", [P, layout["total_ldns"]], f32, isOutput=False
    )
    wt = nc.declare_dram_parameter("wt", [P, P], f32, isOutput=False)
    iotam = nc.declare_dram_parameter("iotam", [P, P], f32, isOutput=False)
    resid = nc.declare_dram_parameter("resid", [npc, P], f32, isOutput=False)
    out = nc.declare_dram_parameter("out", [npc, P], f32, isOutput=True)

    X_BUFS = 3
    with TileContext(nc) as tc:
        with (
            tc.tile_pool(name="const", bufs=1) as constp,
            tc.tile_pool(name="meta", bufs=3) as metap,
            tc.tile_pool(name="x", bufs=X_BUFS) as xp,
            tc.tile_pool(name="s", bufs=6) as sp,
            tc.tile_pool(name="zps", bufs=2, space="PSUM") as zpsp,
            tc.tile_pool(name="yps", bufs=2, space="PSUM") as ypsp,
            tc.tile_pool(name="post", bufs=3) as postp,
        ):
            wt_sb = constp.tile([P, P], f32)
            nc.sync.dma_start(out=wt_sb[:], in_=wt[:, :])
            iota_f = constp.tile([P, P], f32)
            nc.sync.dma_start(out=iota_f[:], in_=iotam[:, :])

            for t in range(n_tiles):
                Ct = int(C_t[t])
                icols = int(icols_tb[t].sum())
                mt_i = metap.tile([P, max(icols, 1)], i16, tag="mi")
                mt_ln = metap.tile([P, 3 * Ct], f32, tag="mldns")
                ic0 = int(icol_off_tile[t])
                nc.sync.dma_start(out=mt_i[:, :icols], in_=idxm[:, ic0 : ic0 + icols])
                lc0 = int(ldns_col_off[t])
                nc.sync.dma_start(out=mt_ln[:], in_=ldns[:, lc0 : lc0 + 3 * Ct])

                # X[e, c*128:(c+1)*128] = feats[gathered src of (chunk c, slot e)]
                X_full = xp.tile([P, Cmax * P], f32, tag="X")
                X = X_full[:, : Ct * P]
                for b in range(nb):
                    n_idx = int(n_tb[t, b])
                    if n_idx == 0:
                        continue
                    co = int(chunk_off_in_tile[t, b])
                    cb = int(ct_tb[t, b])
                    io = int(icol_off_in_tile[t, b])
                    icb = int(icols_tb[t, b])
                    if n_idx % P:
                        # the gather leaves partitions >= n_idx%128 of its
                        # last chunk unwritten; pre-zero that chunk so
                        # 0 * NaN can't poison the one-hot matmul (memzero
                        # bitcasts to uint32 - no NaN read path)
                        nc.scalar.memzero(X[:, (co + cb - 1) * P : (co + cb) * P])
                    nc.gpsimd.dma_gather(
                        out_ap=X[:, co * P : (co + cb) * P].rearrange(
                            "p (c e) -> p c e", e=P
                        ),
                        in_ap=feats[b * B : min((b + 1) * B, n_nodes), :],
                        idxs_ap=mt_i[:, io : io + icb],
                        num_idxs=n_idx,
                        num_idxs_reg=n_idx,
                        elem_size=P,
                        # single_packet concatenates the whole stream into one
                        # SDMA packet; the packet limit is 64 descriptors, and
                        # these calls emit ~70-90 per engine
                        single_packet=False,
                        # one SWDGE queue per bucket: queues run on distinct
                        # Q7 core pairs, parallelizing descriptor generation
                        queue_num=b % 4,
                    )

                z_ps = zpsp.tile([P, P], f32)
                for c in range(Ct):
                    S = sp.tile([P, P], f32, tag="S")
                    # split one-hot builds across DVE and ACT (nc.any piled
                    # all of them onto DVE: 2.9ms busy in the profile).
                    # ACT has no tensor_scalar; for integer iota/ld,
                    # relu(ns - ns*(ld-iota)^2) == (iota==ld)*ns exactly.
                    if c % 2 == 0:
                        nc.vector.tensor_scalar(
                            out=S[:],
                            in0=iota_f[:],
                            scalar1=mt_ln[:, c : c + 1],
                            scalar2=mt_ln[:, Ct + c : Ct + c + 1],
                            op0=mybir.AluOpType.is_equal,
                            op1=mybir.AluOpType.mult,
                        )
                    else:
                        t2 = sp.tile([P, P], f32, tag="T2")
                        nc.scalar.activation(
                            out=t2[:],
                            in_=iota_f[:],
                            func=mybir.ActivationFunctionType.Square,
                            bias=mt_ln[:, c : c + 1],
                            scale=-1.0,
                        )
                        nc.scalar.activation(
                            out=S[:],
                            in_=t2[:],
                            func=mybir.ActivationFunctionType.Relu,
                            bias=mt_ln[:, Ct + c : Ct + c + 1],
                            scale=mt_ln[:, 2 * Ct + c : 2 * Ct + c + 1],
                        )
                    # zT[i, d] += X_c[e, i].T @ S[e, d]
                    nc.tensor.matmul(
                        out=z_ps[:],
                        lhsT=X[:, c * P : (c + 1) * P],
                        rhs=S[:],
                        start=(c == 0),
                        stop=(c == Ct - 1),
                    )

                zT_sb = postp.tile([P, P], f32, tag="zT")
                nc.scalar.copy(out=zT_sb[:], in_=z_ps[:])
                y_ps = ypsp.tile([P, P], f32)
                # y[d, o] = zT[i, d].T @ wt[i, o]
                nc.tensor.matmul(
                    out=y_ps[:], lhsT=zT_sb[:], rhs=wt_sb[:], start=True, stop=True
                )

                rows = P if t < n_tiles - 1 else rows_last
                y_sb = postp.tile([P, P], f32, tag="y")
                nc.scalar.activation(
                    out=y_sb[:], in_=y_ps[:], func=mybir.ActivationFunctionType.Relu
                )
                res_sb = postp.tile([P, P], f32, tag="res")
                nc.sync.dma_start(
                    out=res_sb[:rows], in_=resid[t * P : t * P + rows, :]
                )
                o_sb = postp.tile([P, P], f32, tag="o")
                nc.vector.tensor_add(
                    out=o_sb[:rows], in0=y_sb[:rows], in1=res_sb[:rows]
                )
                nc.sync.dma_start(out=out[t * P : t * P + rows, :], in_=o_sb[:rows])
    nc.finalize()
    return nc


def _run(features, W, edge_src, edge_dst, trace=False, **spmd_kwargs):
    in_maps, layout = _prepare(features, W, edge_src, edge_dst)
    nc = _build_program(layout)
    br = run_bass_kernel_spmd(
        nc, in_maps, core_ids=list(range(N_CORES)), trace=trace, **spmd_kwargs
    )
    outs = [r["out"] for r in br.results]
    full = np.concatenate(outs, axis=0).astype(np.float32)
    return full, br


def kernel(features, W, edge_src, edge_dst):
    out, _ = _run(features, W, edge_src, edge_dst, trace=False)
    return out



# revision 5
# speedup vs baseline: 1.3879x; 1.3879x over previous
"""GCN layer (message passing) on 8 Trainium2 NeuronCores.

out = relu(((D^-1/2 A D^-1/2) X) @ W.T) + X

v2 strategy (bf16 datapath, dst-sharded):
  - Destination nodes sharded across 8 cores (12500 each). Host bakes
    norm[src] into a bf16 feature table (so the one-hot S is binary) and
    applies norm[dst] as the per-partition scale of the final ReLU.
  - Edges sorted by (tile-group of 4 dst tiles, src bucket, dst tile, src).
    One dma_gather per (group, bucket) -> ~100 calls/core instead of 392,
    cutting the serial Q7 SWDGE descriptor-generation time.
  - Per 128-edge chunk: S[e, d] = (ld_e == d) built on DVE (tensor_scalar
    is_equal, bf16) or ACT (two activations, fp32->bf16), alternating to
    balance engines; PE accumulates zT[i, d] += X_c[e, i].T @ S_c[e, d]
    in fp32 PSUM from bf16 operands (no fp32 LOW/HIGH matmul split).
  - Per dst tile: zT -> bf16 SBUF copy, y = zT.T @ W.T on PE,
    relu(norm_dst * y) on ACT (per-partition scale), +residual on DVE.
  - SPMD static sizes: per (group, bucket, tile) segment = max count over
    the 8 cores; pad slots gather row 0 of the bucket and carry ld = -1
    (zero one-hot row). Unwritten tail slots of the last chunk of each
    gather call are memzeroed so 0 * garbage cannot poison PSUM.
"""

import math

import numpy as np
from ml_dtypes import bfloat16

import concourse.bacc as bacc
import concourse.mybir as mybir
from concourse.bass_utils import run_bass_kernel_spmd
from concourse.tile import TileContext

P = 128
N_CORES = 8
BUCKET_MAX = 25000  # int16 gather indices: bucket the node space
GROUP_TILES = 4  # dst tiles per gather group

N_NODES = 100000
NPC = N_NODES // N_CORES  # 12500
N_TILES = math.ceil(NPC / P)  # 98
ROWS_LAST = NPC - (N_TILES - 1) * P  # 84
NB = math.ceil(N_NODES / BUCKET_MAX)  # 4
N_GROUPS = math.ceil(N_TILES / GROUP_TILES)  # 25

# fraction of S builds on DVE (rest on ACT 2-op path)
DVE_EVERY = 3  # op_idx % DVE_EVERY != 0 -> DVE; == 0 -> ACT


def _prepare(features, W, edge_src, edge_dst):
    features = np.asarray(features, dtype=np.float32)
    W = np.asarray(W, dtype=np.float32)
    edge_src = np.asarray(edge_src, dtype=np.int32)
    edge_dst = np.asarray(edge_dst, dtype=np.int32)

    n_nodes, d = features.shape
    assert d == P and n_nodes == N_NODES

    degs = np.bincount(edge_dst, minlength=n_nodes).astype(np.float32)
    norm = 1.0 / np.sqrt(np.maximum(degs, 1.0), dtype=np.float32)
    table = (features * norm[:, None]).astype(bfloat16)  # norm[src] baked in

    core_of = edge_dst // NPC

    # tiles per group (last group may be short)
    gtiles = [
        list(range(g * GROUP_TILES, min((g + 1) * GROUP_TILES, N_TILES)))
        for g in range(N_GROUPS)
    ]

    # per-core sorted edges and per-(group,bucket,tile) counts
    per_core = []
    counts = np.zeros((N_CORES, N_GROUPS, NB, GROUP_TILES), np.int64)
    for k in range(N_CORES):
        sel = np.flatnonzero(core_of == k)
        src_k = edge_src[sel]
        ldst = edge_dst[sel] - k * NPC
        tile = ldst >> 7
        grp = tile // GROUP_TILES
        tin = tile % GROUP_TILES  # tile index within group
        bkt = src_k // BUCKET_MAX
        order = np.lexsort((src_k, tin, bkt, grp))
        src_s = src_k[order]
        grp_s, bkt_s, tin_s = grp[order], bkt[order], tin[order]
        ld_s = (ldst[order] & 127).astype(np.float32)
        gid = (grp_s * NB + bkt_s) * GROUP_TILES + tin_s
        cnt = np.bincount(gid, minlength=N_GROUPS * NB * GROUP_TILES)
        counts[k] = cnt.reshape(N_GROUPS, NB, GROUP_TILES)
        per_core.append((src_s, bkt_s, gid, ld_s))

    seg = counts.max(axis=0)  # [G, NB, GT] static segment sizes

    # ---- static layout ----
    # per (g,b): num_idxs, chunk count, idx col count; offsets
    nidx = seg.sum(axis=2)  # [G, NB]
    ct = (nidx + P - 1) // P
    icb = (nidx + 15) // 16
    co_in_g = np.cumsum(ct, axis=1) - ct  # chunk offset of (g,b) within group
    C_g = ct.sum(axis=1)  # chunks per group
    icols_g = icb.sum(axis=1)
    icol_off_g = np.concatenate([[0], np.cumsum(icols_g)])[:-1]
    icol_off_gb = icol_off_g[:, None] + (np.cumsum(icb, axis=1) - icb)
    total_icols = int(icols_g.sum())
    seg_off = np.cumsum(seg, axis=2) - seg  # slot offset of tile seg in call

    # matmul op program: per group, ordered by (bucket, chunk, tile)
    # op = (chunk_in_group, tile_in_group, s_col_engine, s_col_idx, start, stop)
    ops_per_group = []
    scol_dve = 0
    scol_act = 0
    op_idx = 0
    # first/last op per (g, tin) to set start/stop flags
    for g in range(N_GROUPS):
        ops = []
        for b in range(NB):
            for t, tn in enumerate(gtiles[g]):
                lo = int(seg_off[g, b, t])
                hi = lo + int(seg[g, b, t])
                if hi == lo:
                    continue
                c0, c1 = lo // P, (hi - 1) // P
                for c in range(c0, c1 + 1):
                    r0 = max(lo, c * P) - c * P
                    r1 = min(hi, (c + 1) * P) - c * P
                    use_dve = op_idx % DVE_EVERY != 0
                    if use_dve:
                        sc = scol_dve
                        scol_dve += 1
                    else:
                        sc = scol_act
                        scol_act += 1
                    ops.append(
                        dict(
                            chunk=int(co_in_g[g, b]) + c,
                            tin=t,
                            dve=use_dve,
                            scol=sc,
                            r0=r0,
                            r1=r1,
                            b=b,
                        )
                    )
                    op_idx += 1
        # start/stop flags per tile within group
        seen = {}
        for o in ops:
            if o["tin"] not in seen:
                o["start"] = True
                seen[o["tin"]] = o
            else:
                o["start"] = False
            o["stop"] = False
        last = {}
        for o in ops:
            last[o["tin"]] = o
        for o in last.values():
            o["stop"] = True
        ops_per_group.append(ops)

    n_dve_cols = scol_dve
    n_act_cols = scol_act

    layout = dict(
        gtiles=gtiles,
        seg=seg,
        nidx=nidx,
        ct=ct,
        icb=icb,
        co_in_g=co_in_g,
        C_g=C_g,
        Cmax=int(C_g.max()),
        icol_off_gb=icol_off_gb,
        total_icols=total_icols,
        ops_per_group=ops_per_group,
        n_dve_cols=n_dve_cols,
        n_act_cols=n_act_cols,
    )

    # per-(g,b,t,chunk) -> op column lookup (static, shared by all cores)
    opcol = {}
    for g, ops in enumerate(ops_per_group):
        for o in ops:
            opcol[(g, o["chunk"])] = opcol.get((g, o["chunk"]), {})
            opcol[(g, o["chunk"])][o["tin"]] = (o["dve"], o["scol"])

    wt = np.ascontiguousarray(W.T).astype(bfloat16)  # wt[i, o] = W[o, i]
    iota_bf = np.tile(np.arange(P, dtype=np.float32), (P, 1)).astype(bfloat16)
    iota_f32 = np.tile(np.arange(P, dtype=np.float32), (P, 1))

    in_maps = []
    for k in range(N_CORES):
        src_s, bkt_s, gid, ld_s = per_core[k]
        # position of each edge within its (g,b,t) segment for this core
        cnt_flat = counts[k].reshape(-1)
        gstart = np.zeros(N_GROUPS * NB * GROUP_TILES, np.int64)
        gstart[1:] = np.cumsum(cnt_flat)[:-1]
        pos = np.arange(len(src_s)) - gstart[gid]
        g_of = gid // (NB * GROUP_TILES)
        b_of = (gid // GROUP_TILES) % NB
        t_of = gid % GROUP_TILES
        slot = seg_off[g_of, b_of, t_of] + pos  # slot within the (g,b) call

        idx16 = np.zeros((16, total_icols), np.int16)
        icol = icol_off_gb[g_of, b_of] + slot // 16
        idx16[slot % 16, icol] = (src_s - b_of * BUCKET_MAX).astype(np.int16)
        idxm = np.tile(idx16, (8, 1))

        ld_dve = np.full((P, max(n_dve_cols, 1)), -1.0, np.float32)
        ld_act = np.full((P, max(n_act_cols, 1)), -1.0, np.float32)
        chunk_in_g = co_in_g[g_of, b_of] + slot // P
        row = slot % P
        for j in range(len(src_s)):
            dve, sc = opcol[(g_of[j], chunk_in_g[j])][t_of[j]]
            if dve:
                ld_dve[row[j], sc] = ld_s[j]
            else:
                ld_act[row[j], sc] = ld_s[j]

        normd = np.ones((P, N_TILES), np.float32)
        base = k * NPC
        for t in range(N_TILES):
            rows = P if t < N_TILES - 1 else ROWS_LAST
            normd[:rows, t] = norm[base + t * P : base + t * P + rows]

        in_maps.append(
            {
                "feats": table,
                "idxm": np.ascontiguousarray(idxm),
                "ld_dve": np.ascontiguousarray(ld_dve),
                "ld_act": np.ascontiguousarray(ld_act),
                "wt": wt,
                "iota_bf": iota_bf,
                "iota_f32": iota_f32,
                "normd": normd,
                "resid": np.ascontiguousarray(features[base : base + NPC]),
            }
        )
    return in_maps, layout


def _build_program(layout):
    f32 = mybir.dt.float32
    bf16 = mybir.dt.bfloat16
    i16 = mybir.dt.int16
    gtiles = layout["gtiles"]
    nidx = layout["nidx"]
    ct = layout["ct"]
    icb = layout["icb"]
    co_in_g = layout["co_in_g"]
    C_g = layout["C_g"]
    Cmax = layout["Cmax"]
    icol_off_gb = layout["icol_off_gb"]
    ops_per_group = layout["ops_per_group"]
    n_dve = max(layout["n_dve_cols"], 1)
    n_act = max(layout["n_act_cols"], 1)

    nc = bacc.Bacc(num_swdge_queues=4)
    feats = nc.declare_dram_parameter("feats", [N_NODES, P], bf16, isOutput=False)
    idxm = nc.declare_dram_parameter(
        "idxm", [P, layout["total_icols"]], i16, isOutput=False
    )
    ld_dve_d = nc.declare_dram_parameter("ld_dve", [P, n_dve], f32, isOutput=False)
    ld_act_d = nc.declare_dram_parameter("ld_act", [P, n_act], f32, isOutput=False)
    wt = nc.declare_dram_parameter("wt", [P, P], bf16, isOutput=False)
    iota_bf_d = nc.declare_dram_parameter("iota_bf", [P, P], bf16, isOutput=False)
    iota_f32_d = nc.declare_dram_parameter("iota_f32", [P, P], f32, isOutput=False)
    normd_d = nc.declare_dram_parameter("normd", [P, N_TILES], f32, isOutput=False)
    resid = nc.declare_dram_parameter("resid", [NPC, P], f32, isOutput=False)
    out = nc.declare_dram_parameter("out", [NPC, P], f32, isOutput=True)

    with TileContext(nc) as tc:
        with (
            tc.tile_pool(name="const", bufs=1) as constp,
            tc.tile_pool(name="meta", bufs=3) as metap,
            tc.tile_pool(name="x", bufs=3) as xp,
            tc.tile_pool(name="s", bufs=8) as sp,
            tc.tile_pool(name="zps", bufs=6, space="PSUM") as zpsp,
            tc.tile_pool(name="yps", bufs=2, space="PSUM") as ypsp,
            tc.tile_pool(name="post", bufs=4) as postp,
        ):
            wt_sb = constp.tile([P, P], bf16)
            nc.sync.dma_start(out=wt_sb[:], in_=wt[:, :])
            iota_b = constp.tile([P, P], bf16)
            nc.sync.dma_start(out=iota_b[:], in_=iota_bf_d[:, :])
            iota_f = constp.tile([P, P], f32)
            nc.sync.dma_start(out=iota_f[:], in_=iota_f32_d[:, :])
            # full ld tables stay resident (small)
            ld_dve_sb = constp.tile([P, n_dve], f32)
            nc.sync.dma_start(out=ld_dve_sb[:], in_=ld_dve_d[:, :])
            ld_act_sb = constp.tile([P, n_act], f32)
            nc.sync.dma_start(out=ld_act_sb[:], in_=ld_act_d[:, :])
            normd_sb = constp.tile([P, N_TILES], f32)
            nc.sync.dma_start(out=normd_sb[:], in_=normd_d[:, :])

            for g in range(N_GROUPS):
                icols = int(icb[g].sum())
                mt_i = metap.tile([P, max(icols, 1)], i16, tag="mi")
                ic0 = int(icol_off_gb[g, 0])
                nc.sync.dma_start(out=mt_i[:, :icols], in_=idxm[:, ic0 : ic0 + icols])

                Cg = int(C_g[g])
                X_full = xp.tile([P, Cmax * P], bf16, tag="X")
                X = X_full[:, : Cg * P]
                for b in range(NB):
                    n_idx = int(nidx[g, b])
                    if n_idx == 0:
                        continue
                    co = int(co_in_g[g, b])
                    cb = int(ct[g, b])
                    io = int(icol_off_gb[g, b]) - ic0
                    icbb = int(icb[g, b])
                    if n_idx % P:
                        nc.scalar.memzero(X[:, (co + cb - 1) * P : (co + cb) * P])
                    nc.gpsimd.dma_gather(
                        out_ap=X[:, co * P : (co + cb) * P].rearrange(
                            "p (c e) -> p c e", e=P
                        ),
                        in_ap=feats[
                            b * BUCKET_MAX : min((b + 1) * BUCKET_MAX, N_NODES), :
                        ],
                        idxs_ap=mt_i[:, io : io + icbb],
                        num_idxs=n_idx,
                        num_idxs_reg=n_idx,
                        elem_size=P,
                        single_packet=False,
                        queue_num=b % 4,
                    )

                z_ps = {}
                for o in ops_per_group[g]:
                    tin = o["tin"]
                    if o["start"]:
                        z_ps[tin] = zpsp.tile([P, P], f32, tag="z", name=f"z{g}_{tin}")
                    S = sp.tile([P, P], bf16, tag="S")
                    if o["dve"]:
                        nc.vector.tensor_scalar(
                            out=S[:],
                            in0=iota_b[:],
                            scalar1=ld_dve_sb[:, o["scol"] : o["scol"] + 1],
                            scalar2=None,
                            op0=mybir.AluOpType.is_equal,
                        )
                    else:
                        t2 = sp.tile([P, P], f32, tag="T2")
                        nc.scalar.activation(
                            out=t2[:],
                            in_=iota_f[:],
                            func=mybir.ActivationFunctionType.Square,
                            bias=ld_act_sb[:, o["scol"] : o["scol"] + 1],
                            scale=-1.0,
                        )
                        nc.scalar.activation(
                            out=S[:],
                            in_=t2[:],
                            func=mybir.ActivationFunctionType.Relu,
                            bias=1.0,
                            scale=-1.0,
                        )
                    c = o["chunk"]
                    nc.tensor.matmul(
                        out=z_ps[tin][:],
                        lhsT=X[:, c * P : (c + 1) * P],
                        rhs=S[:],
                        start=o["start"],
                        stop=o["stop"],
                    )

                for tin, tn in enumerate(gtiles[g]):
                    zT_sb = postp.tile([P, P], bf16, tag="zT")
                    nc.scalar.copy(out=zT_sb[:], in_=z_ps[tin][:])
                    y_ps = ypsp.tile([P, P], f32)
                    nc.tensor.matmul(
                        out=y_ps[:], lhsT=zT_sb[:], rhs=wt_sb[:], start=True, stop=True
                    )
                    rows = P if tn < N_TILES - 1 else ROWS_LAST
                    y_sb = postp.tile([P, P], f32, tag="y")
                    nc.scalar.activation(
                        out=y_sb[:],
                        in_=y_ps[:],
                        func=mybir.ActivationFunctionType.Relu,
                        scale=normd_sb[:, tn : tn + 1],
                    )
                    res_sb = postp.tile([P, P], f32, tag="res")
                    nc.sync.dma_start(
                        out=res_sb[:rows], in_=resid[tn * P : tn * P + rows, :]
                    )
                    o_sb = postp.tile([P, P], f32, tag="o")
                    nc.vector.tensor_add(
                        out=o_sb[:rows], in0=y_sb[:rows], in1=res_sb[:rows]
                    )
                    nc.sync.dma_start(
                        out=out[tn * P : tn * P + rows, :], in_=o_sb[:rows]
                    )
    nc.finalize()
    return nc


def _run(features, W, edge_src, edge_dst, trace=False, **spmd_kwargs):
    in_maps, layout = _prepare(features, W, edge_src, edge_dst)
    nc = _build_program(layout)
    br = run_bass_kernel_spmd(
        nc, in_maps, core_ids=list(range(N_CORES)), trace=trace, **spmd_kwargs
    )
    outs = [r["out"] for r in br.results]
    full = np.concatenate(outs, axis=0).astype(np.float32)
    return full, br


def kernel(features, W, edge_src, edge_dst):
    out, _ = _run(features, W, edge_src, edge_dst, trace=False)
    return out
